# revision 21
# baseline (speedup 1.0000x reference)
"""Trainium2 Bass kernel for 2-layer edge-MLP GNN with segment-min aggregation.

Strategy (8 NeuronCores, SPMD, dst-bucket sharding -> no collectives):
- Core k owns nodes [12500k, 12500(k+1)). Within a core, nodes with deg>0
  are sorted by degree (desc) and paired: pair i = (node 2i, node 2i+1) ->
  (stream A = partitions 0:64, stream B = 64:128) of column-slot i.
- Slot i's width = max over cores of deg(A-node i) (sorted profiles are
  near-identical across cores). A node with fewer edges than the slot width
  duplicates one of its edges (min is idempotent). Slots are packed into
  1024-column blocks (PSUM msg tile = 2 banks); a slot never straddles a
  block boundary.
- Blocks have a uniform slot width (profile is monotone non-increasing),
  so the segment-min is exactly ONE vector.tensor_reduce(min) on a
  [128, n, d] view of each 1024-wide PSUM msg tile (amortizes the 125 ns
  PSUM access cost). agg column of a node = its slot ordinal.
- Edge MLP: MM1 (K=12, bf16 hi/lo-split inputs+weights), one wide ACT
  relu (+b1, fp32->bf16) per 1024 cols, MM2 = single bf16 matmul (numerics
  verified: rel ~1e-2 < 2e-2 budget). ab2 folded into update-MLP bias.
- Update MLP in fp16 (bf16-rate matmuls, 10-bit mantissa keeps the final
  stage accurate where bf16 fails); u-relu on the DVE (Act is busier),
  px -> x2 via ACT Copy (same act-table as Relu, avoids a table reload;
  ub2 added on host), per-block DMA out.
- Engine balance per layer (TimelineSim, matches quiet-HW within 2%):
  DVE ~126 us (segment-min, the roofline), ACT ~112 us (relu),
  PE ~95 us (matmuls). Makespan ~144 us/layer.
- One compiled program, launched once per layer; host stages x[src] rows
  between launches (inter-layer gather + unpack are host-side).
"""

import math

import numpy as np
import ml_dtypes

import concourse.bass as bass
import concourse.bacc as bacc
import concourse.mybir as mybir
import concourse.tile as tile
from concourse.bass_utils import run_bass_kernel_spmd

F32 = mybir.dt.float32
F32R = mybir.dt.float32r
F16 = mybir.dt.float16
BF16 = mybir.dt.bfloat16

N_NODES = 100000
N_EDGES = 1600000
N_CORES = 8
NODES_PER_CORE = N_NODES // N_CORES
HID = 64
BLOCK = 1024  # msg/pre PSUM tile width (2 banks)
CHUNK = 8     # rhs tiles (of 512) per staging DMA


def _bf(a):
    return a.astype(ml_dtypes.bfloat16).astype(np.float32)


def _split_hi_lo(a):
    hi = _bf(a)
    return hi, a - hi


# ----------------------------------------------------------------------------
# Host-side layout construction (shared compiled structure across cores)
# ----------------------------------------------------------------------------

def build_layout(edge_index):
    src = np.asarray(edge_index[0], np.int64)
    dst = np.asarray(edge_index[1], np.int64)
    deg = np.bincount(dst, minlength=N_NODES)

    # CSR over edges by dst
    order = np.argsort(dst, kind="stable")
    starts = np.searchsorted(dst[order], np.arange(N_NODES))

    # per-core degree-desc sorted nonzero nodes, paired into streams A/B
    nodesA = []
    nodesB = []
    for k in range(N_CORES):
        nk = np.arange(k * NODES_PER_CORE, (k + 1) * NODES_PER_CORE)
        nz = nk[deg[nk] > 0]
        o = np.argsort(-deg[nz], kind="stable")
        snodes = nz[o]
        nodesA.append(snodes[0::2])
        nodesB.append(snodes[1::2])
    P = max(len(a) for a in nodesA)
    nA = np.full((N_CORES, P), -1, np.int64)
    nB = np.full((N_CORES, P), -1, np.int64)
    for k in range(N_CORES):
        nA[k, :len(nodesA[k])] = nodesA[k]
        nB[k, :len(nodesB[k])] = nodesB[k]
    degA = np.where(nA >= 0, deg[np.maximum(nA, 0)], 0)
    slotdeg = degA.max(axis=0)  # [P] width of each slot (desc-ish)
    assert slotdeg.min() >= 1

    # uniform-width blocks: slotdeg is non-increasing, so the width of a
    # block is its first slot's degree; every slot in the block is padded to
    # that width (duplicate edges; min is idempotent). Exactly one reduce
    # instruction per block.
    blk = np.zeros(P, np.int64)
    col0 = np.zeros(P, np.int64)
    width = np.zeros(P, np.int64)
    b, cur = 0, 0
    dblk = int(slotdeg[0])
    for i in range(P):
        if cur + dblk > BLOCK:
            b += 1
            cur = 0
            dblk = int(slotdeg[i])
        width[i] = dblk
        blk[i] = b
        col0[i] = b * BLOCK + cur
        cur += dblk
    NBLK = b + 1
    L = NBLK * BLOCK
    NT = L // 512

    # one run per block
    runs = [[] for _ in range(NBLK)]
    i = 0
    while i < P:
        j = i
        while j + 1 < P and blk[j + 1] == blk[i]:
            j += 1
        runs[blk[i]].append((0, int(j - i + 1), int(width[i]), int(i)))
        i = j + 1
    slotdeg = width

    # slot -> edge-id assignment, vectorized per (core, stream)
    tot = int(slotdeg.sum())
    slot_rep = np.repeat(np.arange(P), slotdeg)
    base = np.repeat(np.concatenate([[0], np.cumsum(slotdeg)[:-1]]), slotdeg)
    off = np.arange(tot) - base          # 0..slotdeg[i)-1 within slot
    pos = np.repeat(col0, slotdeg) + off  # global column

    slot_edge = np.zeros((N_CORES, 2, L), np.int64)
    node_pos = np.full((N_NODES, 2), -1, np.int64)
    for k in range(N_CORES):
        fill_edge = order[starts[nodesA[k][0]]]
        slot_edge[k, :, :] = fill_edge
        for s, nodes in ((0, nA[k]), (1, nB[k])):
            nd = nodes[slot_rep]
            valid = nd >= 0
            ndv = nd[valid]
            j = np.minimum(off[valid], deg[ndv] - 1)
            eids = order[starts[ndv] + j]
            slot_edge[k, s, pos[valid]] = eids
            real = nodes >= 0
            node_pos[nodes[real], 0] = s
            node_pos[nodes[real], 1] = np.arange(P)[real]

    C = P
    UB = (C + BLOCK - 1) // BLOCK
    C_pad = UB * BLOCK
    zero_nodes = np.where(deg == 0)[0]
    return dict(
        NBLK=NBLK, L=L, NT=NT, C=C, C_pad=C_pad, UB=UB, runs=runs,
        slot_edge=slot_edge, node_pos=node_pos, zero_nodes=zero_nodes,
        src=src, dst=dst,
    )


def build_rhs(layout, x_full, edge_attr):
    """Per-core rhs [12, L] bf16; rows per stream s:
    [xhi, xhi, xlo, ehi, ehi, elo] at rows 6s..6s+5."""
    L = layout["L"]
    src = layout["src"]
    rhs = np.zeros((N_CORES, 12, L), np.float32)
    for k in range(N_CORES):
        for s in range(2):
            eids = layout["slot_edge"][k, s]
            xv = x_full[src[eids]]
            ev = edge_attr[eids]
            xhi, xlo = _split_hi_lo(xv)
            ehi, elo = _split_hi_lo(ev)
            r0 = 6 * s
            rhs[k, r0 + 0] = xhi
            rhs[k, r0 + 1] = xhi
            rhs[k, r0 + 2] = xlo
            rhs[k, r0 + 3] = ehi
            rhs[k, r0 + 4] = ehi
            rhs[k, r0 + 5] = elo
    return rhs.astype(ml_dtypes.bfloat16)


def build_weights(aW1, ab1, aW2, ab2, uW1, ub1, uW2, ub2):
    """Pack one layer's weights for the compiled program."""
    # MM1 lhsT [12, 128] bf16: per scalar v rows [vh*wh, vh*wl, vl*wh]
    w1 = np.zeros((12, 128), np.float32)
    for s in range(2):
        c0 = 64 * s
        r0 = 6 * s
        for scalar_i in range(2):  # x then e
            w = aW1[scalar_i]  # [64]
            wh, wl = _split_hi_lo(w)
            w1[r0 + 3 * scalar_i + 0, c0:c0 + 64] = wh
            w1[r0 + 3 * scalar_i + 1, c0:c0 + 64] = wl
            w1[r0 + 3 * scalar_i + 2, c0:c0 + 64] = wh
    # MM2 lhsT blockdiag single bf16 [128, 128]
    w2 = np.zeros((128, 128), np.float32)
    for s in range(2):
        w2[64 * s:64 * s + 64, 64 * s:64 * s + 64] = aW2
    # biases
    b1vec = np.concatenate([ab1, ab1]).reshape(128, 1).astype(np.float32)
    # fold ab2 into ub1: ub1' = uW1.T @ ab2 + ub1
    ub1f = (uW1.T @ ab2 + ub1).astype(np.float32)
    ub1vec = np.concatenate([ub1f, ub1f]).reshape(128, 1).astype(np.float32)
    uw1blk = np.zeros((128, 128), np.float32)
    uw1blk[:64, :64] = uW1
    uw1blk[64:, 64:] = uW1
    uw2blk = np.zeros((128, 2), np.float32)
    uw2blk[:64, 0] = uW2[:, 0]
    uw2blk[64:, 1] = uW2[:, 0]
    ub2vec = np.array([[ub2[0]], [ub2[0]]], np.float32)
    return dict(
        w1=w1.astype(ml_dtypes.bfloat16),
        w2=w2.astype(ml_dtypes.bfloat16),
        b1vec=b1vec, ub1vec=ub1vec,
        uw1blk=uw1blk.astype(np.float16), uw2blk=uw2blk.astype(np.float16),
        ub2vec=ub2vec,
    )


# ----------------------------------------------------------------------------
# Bass program (compiled once; same structure for all cores and both layers)
# ----------------------------------------------------------------------------

def build_program(layout, bench_reps=1, skip=(), fp16_update=True,
                  hp_bufs=3, st_bufs=3, pre_bufs=2, msg_bufs=2,
                  interleave_update=False, upd_pools=("pre", "msg"),
                  u_on_dve=False, px_copy=False):
    skip = set(skip)
    AGG_DT = F16 if fp16_update else F32
    NBLK, L, NT = layout["NBLK"], layout["L"], layout["NT"]
    UB, C_pad = layout["UB"], layout["C_pad"]
    runs = layout["runs"]

    # block index at which each update-block's agg columns are complete
    slot_blk = []
    for bi, rr in enumerate(runs):
        for (c0, n, dd, ac0) in rr:
            slot_blk.extend([bi] * n)
    C = layout["C"]
    ready_at = {}
    for ui in range(UB):
        last_slot = min((ui + 1) * BLOCK, C) - 1
        ready_at.setdefault(slot_blk[last_slot], []).append(ui)

    nc = bacc.Bacc("TRN2", target_bir_lowering=False, debug=False,
                   num_devices=N_CORES)
    rhs_d = nc.dram_tensor("rhs", [12, L], BF16, kind="ExternalInput")
    w1_d = nc.dram_tensor("w1", [12, 128], BF16, kind="ExternalInput")
    w2_d = nc.dram_tensor("w2", [128, 128], BF16, kind="ExternalInput")
    b1_d = nc.dram_tensor("b1v", [128, 1], F32, kind="ExternalInput")
    ub1_d = nc.dram_tensor("ub1v", [128, 1], F32, kind="ExternalInput")
    uw1_d = nc.dram_tensor("uw1", [128, 128], AGG_DT, kind="ExternalInput")
    uw2_d = nc.dram_tensor("uw2", [128, 2], AGG_DT, kind="ExternalInput")
    ub2_d = nc.dram_tensor("ub2v", [2, 1], F32, kind="ExternalInput")
    x2_d = nc.dram_tensor("x2out", [2, C_pad], F32, kind="ExternalOutput")

    with tile.TileContext(nc) as tc:
        with (
            tc.tile_pool(name="const", bufs=1) as constp,
            tc.tile_pool(name="stage", bufs=st_bufs) as stagep,
            tc.tile_pool(name="hpool", bufs=hp_bufs) as hp,
            tc.tile_pool(name="aggp", bufs=1) as aggp,
            tc.tile_pool(name="upool", bufs=2) as up,
            tc.tile_pool(name="x2p", bufs=1) as x2p,
            tc.tile_pool(name="prep", bufs=pre_bufs, space="PSUM") as prep,
            tc.tile_pool(name="msgp", bufs=msg_bufs, space="PSUM") as msgp,
        ):
            # DMA order = first-use order: w1/b1 gate block 0, then the
            # first rhs chunk (persistent buffer; rhs is constant across
            # reps), then w2, then the update-MLP constants (first needed
            # ~130 us in).
            w1_t = constp.tile([12, 128], BF16)
            nc.sync.dma_start(w1_t[:], w1_d[:, :])
            b1_t = constp.tile([128, 1], F32)
            nc.sync.dma_start(b1_t[:], b1_d[:, :])
            st0 = constp.tile([12, CHUNK * 512], BF16)
            ct0 = min(CHUNK, NT)
            nc.sync.dma_start(st0[:, :2 * 512], rhs_d[:, :2 * 512])
            w2_t = constp.tile([128, 128], BF16)
            nc.sync.dma_start(w2_t[:], w2_d[:, :])
            for p0 in range(2 * 512, ct0 * 512, 3 * 512):
                pw = min(3 * 512, ct0 * 512 - p0)
                nc.sync.dma_start(st0[:, p0:p0 + pw], rhs_d[:, p0:p0 + pw])
            uw1_t = constp.tile([128, 128], AGG_DT)
            nc.sync.dma_start(uw1_t[:], uw1_d[:, :])
            uw2_t = constp.tile([128, 2], AGG_DT)
            nc.sync.dma_start(uw2_t[:], uw2_d[:, :])
            ub1_t = constp.tile([128, 1], F32)
            nc.sync.dma_start(ub1_t[:], ub1_d[:, :])
            ub2_t = constp.tile([2, 1], F32)
            nc.sync.dma_start(ub2_t[:], ub2_d[:, :])

            agg_t = aggp.tile([128, C_pad], AGG_DT)
            x2_t = x2p.tile([2, C_pad], F32)

            import contextlib

            pool_by = {"pre": (prep, "pre"), "msg": (msgp, "msg")}

            def emit_update(ui):
                o = ui * BLOCK
                w = min(BLOCK, C - o)      # real columns in this ublock
                pup, putag = pool_by[upd_pools[0]]
                pxp, pxtag = pool_by[upd_pools[1]]
                pu = pup.tile([128, BLOCK], F32, tag=putag)
                for c0 in range(0, w, 512):
                    cw = min(512, w - c0)
                    nc.tensor.matmul(pu[:, c0:c0 + cw], uw1_t[:],
                                     agg_t[:, o + c0:o + c0 + cw],
                                     start=True, stop=True)
                u_t = up.tile([128, BLOCK], AGG_DT, tag="u")
                if u_on_dve:
                    nc.vector.tensor_scalar(
                        u_t[:, :w], pu[:, :w], ub1_t[:], 0.0,
                        op0=mybir.AluOpType.add, op1=mybir.AluOpType.max)
                else:
                    nc.scalar.activation(u_t[:, :w], pu[:, :w],
                                         mybir.ActivationFunctionType.Relu,
                                         bias=ub1_t[:], scale=1.0)
                px = pxp.tile([2, BLOCK], F32, tag=pxtag)
                for c0 in range(0, w, 512):
                    cw = min(512, w - c0)
                    nc.tensor.matmul(px[:, c0:c0 + cw], uw2_t[:],
                                     u_t[:, c0:c0 + cw],
                                     start=True, stop=True)
                if px_copy:
                    nc.scalar.activation(
                        x2_t[:, o:o + w], px[:, :w],
                        mybir.ActivationFunctionType.Copy,
                        bias=0.0, scale=1.0)
                else:
                    nc.scalar.activation(
                        x2_t[:, o:o + w], px[:, :w],
                        mybir.ActivationFunctionType.Identity,
                        bias=ub2_t[:], scale=1.0)
                nc.sync.dma_start(x2_d[:, o:o + w],
                                  x2_t[:, o:o + w])

            loop_cm = tc.For_i(0, bench_reps) if bench_reps > 1 \
                else contextlib.nullcontext()
            with loop_cm:
                emitted_ui = set()
                # ---- edge pipeline (software-pipelined by one block) ----
                h_hist = [None, None]  # h tiles of recent blocks
                st = None
                for b in range(NBLK + 1):
                    if b >= 1 and 'mm2' not in skip:
                        hprev = h_hist[(b - 1) % 2]
                        msg = msgp.tile([128, BLOCK], F32, tag="msg")
                        nc.tensor.matmul(msg[:, 0:512], w2_t[:],
                                         hprev[:, 0:512],
                                         start=True, stop=True)
                        nc.tensor.matmul(msg[:, 512:1024], w2_t[:],
                                         hprev[:, 512:1024],
                                         start=True, stop=True)
                        for (c0, n, d, ac0) in ([] if 'reduce' in skip
                                                else runs[b - 1]):
                            nc.vector.tensor_reduce(
                                agg_t[:, ac0:ac0 + n],
                                msg[:, c0:c0 + n * d].rearrange(
                                    "p (n d) -> p n d", d=d),
                                axis=mybir.AxisListType.X,
                                op=mybir.AluOpType.min)
                        if (interleave_update and 'update' not in skip
                                and 'reduce' not in skip):
                            for ui in ready_at.get(b - 1, []):
                                if b - 1 < NBLK - 1:
                                    emit_update(ui)
                                    emitted_ui.add(ui)
                    if b < NBLK:
                        t0 = 2 * b
                        if t0 % CHUNK == 0:
                            c = t0 // CHUNK
                            if c == 0:
                                st = st0   # preloaded once; rhs constant
                            else:
                                ct = min(CHUNK, NT - c * CHUNK)
                                st = stagep.tile([12, CHUNK * 512], BF16,
                                                 tag="st")
                                nc.sync.dma_start(
                                    st[:, :ct * 512],
                                    rhs_d[:, c * CHUNK * 512:
                                          (c * CHUNK + ct) * 512])
                        j0 = t0 % CHUNK
                        pre = prep.tile([128, BLOCK], F32, tag="pre")
                        nc.tensor.matmul(pre[:, 0:512], w1_t[:],
                                         st[:, j0 * 512:(j0 + 1) * 512],
                                         start=True, stop=True)
                        nc.tensor.matmul(pre[:, 512:1024], w1_t[:],
                                         st[:, (j0 + 1) * 512:(j0 + 2) * 512],
                                         start=True, stop=True)
                    if b < NBLK and 'act' not in skip:
                        h_t = hp.tile([128, BLOCK], BF16, tag="h")
                        nc.scalar.activation(h_t[:], pre[:],
                                             mybir.ActivationFunctionType.Relu,
                                             bias=b1_t[:], scale=1.0)
                        h_hist[b % 2] = h_t

                # ---- update MLP (tail; skipped per-ui if interleaved) ----
                for ui in range(UB if 'update' not in skip else 0):
                    if ui in emitted_ui:
                        continue
                    emit_update(ui)
    nc.compile()
    return nc


def _update_zero_nodes(x_next, zero_nodes, uW1, ub1, uW2, ub2, ab2):
    if len(zero_nodes) == 0:
        return
    # agg = 0 (+ folded ab2): u = relu(uW1.T @ ab2 + ub1); x = uW2.T u + ub2
    u = np.maximum(uW1.T @ ab2 + ub1, 0.0)
    x_val = float(uW2[:, 0] @ u + ub2[0])
    x_next[zero_nodes] = x_val


def kernel(x, edge_attr, aW1, ab1, aW2, ab2, uW1, ub1, uW2, ub2, edge_index):
    x = np.asarray(x, np.float32)
    edge_attr = np.asarray(edge_attr, np.float32)
    edge_index = np.asarray(edge_index)
    aW1 = np.asarray(aW1, np.float32); ab1 = np.asarray(ab1, np.float32)
    aW2 = np.asarray(aW2, np.float32); ab2 = np.asarray(ab2, np.float32)
    uW1 = np.asarray(uW1, np.float32); ub1 = np.asarray(ub1, np.float32)
    uW2 = np.asarray(uW2, np.float32); ub2 = np.asarray(ub2, np.float32)

    layout = build_layout(edge_index)
    nc = build_program(layout, u_on_dve=True, px_copy=True)

    x_cur = x[:, 0].copy()
    ea = edge_attr[:, 0]
    node_pos = layout["node_pos"]
    mapped = node_pos[:, 0] >= 0
    core_of_node = np.arange(N_NODES) // NODES_PER_CORE

    for l in range(2):
        wts = build_weights(aW1[l], ab1[l], aW2[l], ab2[l],
                            uW1[l], ub1[l], uW2[l], ub2[l])
        rhs = build_rhs(layout, x_cur, ea)
        in_maps = []
        for k in range(N_CORES):
            m = {"rhs": np.asarray(rhs[k]),
                 "w1": wts["w1"], "w2": wts["w2"],
                 "b1v": wts["b1vec"], "ub1v": wts["ub1vec"],
                 "uw1": wts["uw1blk"], "uw2": wts["uw2blk"],
                 "ub2v": wts["ub2vec"]}
            in_maps.append(m)
        res = run_bass_kernel_spmd(nc, in_maps, core_ids=list(range(N_CORES)),
                                   trace=False)
        x_next = np.zeros(N_NODES, np.float32)
        for k in range(N_CORES):
            out_k = res.results[k]["x2out"]  # [2, C_pad]
            sel = mapped & (core_of_node == k)
            ids = np.where(sel)[0]
            # device omits ub2 (px uses a plain Copy); add it here
            x_next[ids] = out_k[node_pos[ids, 0], node_pos[ids, 1]] \
                + ub2[l, 0]
        _update_zero_nodes(x_next, layout["zero_nodes"],
                           uW1[l], ub1[l], uW2[l], ub2[l], ab2[l])
        x_cur = x_next

    return x_cur[:, None].astype(np.float32)
